# revision 1
# baseline (speedup 1.0000x reference)
"""Trainium2 Bass kernel for single-query attention over per-sample concepts.

    sab[b, k] = (query[b] . concept[b, k]) / sqrt(D)
    score     = softmax(sab, axis=-1)
    out[b]    = sum_k score[b, k] * concept[b, k]

Shapes: query [256, 1024] f32, concept [256, 2048, 1024] f32 -> out [256, 1024].

Sharding: pure data parallel, batch 256 split as 32 samples on each of 8
NeuronCores. Memory-bound: each core streams its 256 MiB concept shard once.

Per-core dataflow, per sample b (all tiles [128 k-partitions, 1024 d-free]):
  - qb = broadcast(query[b]) to 128 partitions (GPSIMD partition_broadcast)
  - DMA c-tile t (128 k's), alternating the SP/ACT HWDGE rings (dual-ring
    issue lifts sustained HBM bandwidth ~370 -> ~395 GB/s)
  - DVE scalar_tensor_tensor: elementwise (c*scale)*qb with accum_out
    -> raw scores s[128, 1] per tile (fused multiply+reduce, one pass)
  - ACT exp per tile -> e[128, 1]
  - PE matmul: acc[1, 0:512] += e_t.T @ c_t[:, 0:512], same for 512:1024
    (fp32, PSUM accumulate over the 16 k-tiles)
  - denominator: ACT copy of e-columns with accum_out -> per-partition sums,
    then PE matmul with ones stationary -> [1, 1] in PSUM
  - DVE reciprocal, ACT Copy-with-scale to normalize, DMA out row.
"""

import numpy as np
from contextlib import ExitStack

import concourse.bacc as bacc
import concourse.tile as tile
from concourse import mybir
from concourse.bass_utils import run_bass_kernel_spmd

B, K, D = 256, 2048, 1024
NCORES = 8
BL = B // NCORES          # 32 samples per core
KT = 128                  # k-tile size (partition dim)
NT = K // KT              # 16 k-tiles per sample
SCALE = 1.0 / float(np.sqrt(D))

_cache = {}


def build_nc():
    nc = bacc.Bacc("TRN2", target_bir_lowering=False, debug=False,
                   num_devices=NCORES)
    q = nc.dram_tensor("query", [BL, D], mybir.dt.float32, kind="ExternalInput")
    c = nc.dram_tensor("concept", [BL, K, D], mybir.dt.float32r,
                       kind="ExternalInput")
    out = nc.dram_tensor("out", [BL, D], mybir.dt.float32,
                         kind="ExternalOutput")
    f32 = mybir.dt.float32

    f32r = mybir.dt.float32r

    with tile.TileContext(nc) as tc, ExitStack() as ctx:
        cpool = ctx.enter_context(tc.tile_pool(name="c", bufs=16))
        qpool = ctx.enter_context(tc.tile_pool(name="q", bufs=3))
        spool = ctx.enter_context(tc.tile_pool(name="scr", bufs=2))
        epool = ctx.enter_context(tc.tile_pool(name="e", bufs=3))
        onepool = ctx.enter_context(tc.tile_pool(name="one", bufs=1))
        opool = ctx.enter_context(tc.tile_pool(name="o", bufs=4))
        ppool = ctx.enter_context(tc.tile_pool(name="ps", bufs=2, space="PSUM"))
        dpool = ctx.enter_context(tc.tile_pool(name="dn", bufs=2, space="PSUM"))

        ones = onepool.tile([KT, 1], f32)
        nc.vector.memset(ones[:], 1.0)

        for b in range(BL):
            qrow = qpool.tile([1, D], f32)
            nc.scalar.dma_start(out=qrow[:], in_=q[b : b + 1, :])
            qb = qpool.tile([KT, D], f32)
            nc.gpsimd.partition_broadcast(qb[:], qrow[:])

            scols = epool.tile([KT, NT], f32)
            ecols = epool.tile([KT, NT], f32r)
            acc_lo = ppool.tile([1, 512], f32)
            acc_hi = ppool.tile([1, 512], f32)

            for t in range(NT):
                ct = cpool.tile([KT, D], f32r)
                # alternate the two HWDGE rings (SP / ACT) for issue overlap
                dma_eng = nc.sync if t % 2 == 0 else nc.scalar
                dma_eng.dma_start(out=ct[:], in_=c[b, t * KT : (t + 1) * KT, :])
                scr = spool.tile([KT, D], f32)
                nc.vector.scalar_tensor_tensor(
                    out=scr[:],
                    in0=ct[:].bitcast(f32),
                    scalar=SCALE,
                    in1=qb[:],
                    op0=mybir.AluOpType.mult,
                    op1=mybir.AluOpType.mult,
                    accum_out=scols[:, t : t + 1],
                )
                nc.scalar.activation(
                    out=ecols[:, t : t + 1],
                    in_=scols[:, t : t + 1],
                    func=mybir.ActivationFunctionType.Exp,
                )
                e_t = ecols[:, t : t + 1]
                nc.tensor.matmul(acc_lo[:], e_t, ct[:, 0:512],
                                 start=(t == 0), stop=(t == NT - 1))
                nc.tensor.matmul(acc_hi[:], e_t, ct[:, 512:1024],
                                 start=(t == 0), stop=(t == NT - 1))

            # denominator: per-partition sums of e, then reduce across
            # partitions with a ones-stationary matmul
            ered = epool.tile([KT, 1], f32)
            escr = spool.tile([KT, NT], f32)
            nc.scalar.activation(
                out=escr[:],
                in_=ecols[:].bitcast(f32),
                func=mybir.ActivationFunctionType.Copy,
                accum_out=ered[:],
            )
            denom = dpool.tile([1, 1], f32)
            nc.tensor.matmul(denom[:], ones[:], ered[:], start=True, stop=True)

            recip = opool.tile([1, 1], f32)
            nc.vector.reciprocal(recip[:], denom[:])

            orow = opool.tile([1, D], f32)
            nc.scalar.activation(out=orow[:, 0:512], in_=acc_lo[:],
                                 func=mybir.ActivationFunctionType.Copy,
                                 scale=recip[:])
            nc.scalar.activation(out=orow[:, 512:1024], in_=acc_hi[:],
                                 func=mybir.ActivationFunctionType.Copy,
                                 scale=recip[:])
            nc.scalar.dma_start(out=out[b : b + 1, :], in_=orow[:])

    nc.compile()
    return nc


def _run(query, concept, trace=False, trace_kwargs=None):
    if "nc" not in _cache:
        _cache["nc"] = build_nc()
    nc = _cache["nc"]
    in_maps = []
    for i in range(NCORES):
        in_maps.append({
            "query": np.ascontiguousarray(query[i * BL : (i + 1) * BL]),
            "concept": np.ascontiguousarray(concept[i * BL : (i + 1) * BL]),
        })
    res = run_bass_kernel_spmd(
        nc, in_maps, core_ids=list(range(NCORES)),
        trace=trace, **(trace_kwargs or {}),
    )
    out = np.concatenate([res.results[i]["out"] for i in range(NCORES)], axis=0)
    return out.astype(np.float32), res


def kernel(query: np.ndarray, concept: np.ndarray) -> np.ndarray:
    out, _ = _run(np.asarray(query, np.float32), np.asarray(concept, np.float32))
    return out



# revision 7
# speedup vs baseline: 1.0397x; 1.0397x over previous
"""Trainium2 Bass kernel for single-query attention over per-sample concepts.

    sab[b, k] = (query[b] . concept[b, k]) / sqrt(D)
    score     = softmax(sab, axis=-1)
    out[b]    = sum_k score[b, k] * concept[b, k]

Shapes: query [256, 1024] f32, concept [256, 2048, 1024] f32 -> out [256, 1024].

Sharding: pure data parallel, batch 256 split as 32 samples on each of 8
NeuronCores. Memory-bound: the tolerance (2e-2) admits streaming concept in
bf16, halving HBM traffic to 128 MiB per core (measured end-to-end rel err
~3e-3). Host converts to bf16 and pre-folds the 1/sqrt(D) scale into query.

Per-core dataflow (chunk = [128 part, 8, 1024] bf16 = 2 MiB, partition p
holds k-rows 8p..8p+7 of a 1024-row block; 2 chunks per sample):
  - DMA chunk (contiguous 16 KiB per partition), alternating SP/ACT HWDGE
  - qb = partition_broadcast(query row); TT in1 uses a stride-0 AP so one
    2x-mode (bf16) tensor_tensor computes prod = chunk * qb for all 8 rows
  - per row j: free-dim reduce of prod[:, j] -> raw score; split between
    ACT (activation Copy + accum_out) and DVE (tensor_scalar + accum_out,
    4x mode) so neither engine exceeds the DMA budget
  - one ACT Exp per chunk: scols -> ecols (bf16) + accum_out denominator part
  - PE: acc[1, 512] += e_col.T @ chunk[:, j, half] (bf16, PSUM accumulate
    over 16 rows x 2 halves)
  - denom = ones.T @ (ered0 + ered1); DVE reciprocal; ACT Copy-with-scale
    normalizes; DMA out row.
"""

import numpy as np
import ml_dtypes
from contextlib import ExitStack

import concourse.bacc as bacc
import concourse.tile as tile
from concourse import mybir
from concourse.bass_utils import run_bass_kernel_spmd

B, K, D = 256, 2048, 1024
NCORES = 8
BL = B // NCORES          # 32 samples per core
RPP = 8                   # k-rows per partition within a chunk
CH_ROWS = 128 * RPP       # 1024 k-rows per chunk
NCH = K // CH_ROWS        # 2 chunks per sample
SCALE = 1.0 / float(np.sqrt(D))

# Reduce-slice assignment per chunk: first ACT_RED rows reduced on the ACT
# engine, the rest on DVE tensor_scalar (4x mode).
ACT_RED = 5
# Query-broadcast mode: "bcast" = stride-0 AP into the tensor_tensor,
# "copies" = materialize qb8 with doubling DVE copies.
QB8_MODE = "bcast"

_cache = {}


def build_nc(bl=BL, act_red=None, qb8_mode=None):
    act_red = ACT_RED if act_red is None else act_red
    qb8_mode = QB8_MODE if qb8_mode is None else qb8_mode
    nc = bacc.Bacc("TRN2", target_bir_lowering=False, debug=False,
                   num_devices=NCORES)
    f32 = mybir.dt.float32
    bf16 = mybir.dt.bfloat16

    q = nc.dram_tensor("query", [bl, D], bf16, kind="ExternalInput")
    c = nc.dram_tensor("concept", [bl, NCH, 128, RPP, D], bf16,
                       kind="ExternalInput")
    out = nc.dram_tensor("out", [bl, D], f32, kind="ExternalOutput")

    with tile.TileContext(nc) as tc, ExitStack() as ctx:
        cpool = ctx.enter_context(tc.tile_pool(name="c", bufs=4))
        prpool = ctx.enter_context(tc.tile_pool(name="pr", bufs=2))
        qpool = ctx.enter_context(tc.tile_pool(name="q", bufs=3))
        spool = ctx.enter_context(tc.tile_pool(name="s", bufs=3))
        onepool = ctx.enter_context(tc.tile_pool(name="one", bufs=1))
        opool = ctx.enter_context(tc.tile_pool(name="o", bufs=4))
        plo = ctx.enter_context(tc.tile_pool(name="plo", bufs=2, space="PSUM"))
        phi = ctx.enter_context(tc.tile_pool(name="phi", bufs=2, space="PSUM"))
        dpool = ctx.enter_context(tc.tile_pool(name="dn", bufs=2, space="PSUM"))

        ones = onepool.tile([128, 1], f32)
        nc.vector.memset(ones[:], 1.0)

        for b in range(bl):
            qrow = qpool.tile([1, D], bf16)
            nc.sync.dma_start(out=qrow[:], in_=q[b : b + 1, :])
            qb = qpool.tile([128, D], bf16)
            nc.gpsimd.partition_broadcast(qb[:], qrow[:])
            if qb8_mode == "bcast":
                qb_in1 = qb[:].unsqueeze(1).broadcast_to([128, RPP, D])
            else:
                qb8 = qpool.tile([128, RPP, D], bf16)
                nc.vector.tensor_copy(out=qb8[:, 0], in_=qb[:])
                nc.vector.tensor_copy(out=qb8[:, 1], in_=qb8[:, 0])
                nc.vector.tensor_copy(out=qb8[:, 2:4], in_=qb8[:, 0:2])
                nc.vector.tensor_copy(out=qb8[:, 4:8], in_=qb8[:, 0:4])
                qb_in1 = qb8[:]

            scols = spool.tile([128, NCH * RPP], f32)
            ecols = spool.tile([128, NCH * RPP], bf16)
            ered = spool.tile([128, NCH], f32)
            acc_lo = plo.tile([1, 512], f32)
            acc_hi = phi.tile([1, 512], f32)

            for h in range(NCH):
                ct = cpool.tile([128, RPP, D], bf16)
                dma_eng = nc.sync if h % 2 == 0 else nc.scalar
                dma_eng.dma_start(out=ct[:], in_=c[b, h])

                prod = prpool.tile([128, RPP, D], bf16)
                nc.vector.tensor_tensor(out=prod[:], in0=ct[:], in1=qb_in1,
                                        op=mybir.AluOpType.mult)

                for j in range(RPP):
                    col = h * RPP + j
                    sl = prod[:, j]
                    acc = scols[:, col : col + 1]
                    if j < act_red:
                        nc.scalar.activation(
                            out=sl, in_=sl,
                            func=mybir.ActivationFunctionType.Copy,
                            accum_out=acc,
                        )
                    else:
                        nc.vector.tensor_scalar(
                            out=sl, in0=sl, scalar1=1.0, scalar2=0.0,
                            op0=mybir.AluOpType.mult, op1=mybir.AluOpType.add,
                            accum_out=acc,
                        )

                nc.scalar.activation(
                    out=ecols[:, h * RPP : (h + 1) * RPP],
                    in_=scols[:, h * RPP : (h + 1) * RPP],
                    func=mybir.ActivationFunctionType.Exp,
                    accum_out=ered[:, h : h + 1],
                )

                for j in range(RPP):
                    col = h * RPP + j
                    e = ecols[:, col : col + 1]
                    nc.tensor.matmul(acc_lo[:], e, ct[:, j, 0:512],
                                     start=(col == 0),
                                     stop=(col == NCH * RPP - 1))
                    nc.tensor.matmul(acc_hi[:], e, ct[:, j, 512:1024],
                                     start=(col == 0),
                                     stop=(col == NCH * RPP - 1))

            esum = spool.tile([128, 1], f32)
            nc.vector.tensor_add(esum[:], ered[:, 0:1], ered[:, 1:2])
            denom = dpool.tile([1, 1], f32)
            nc.tensor.matmul(denom[:], ones[:], esum[:], start=True, stop=True)

            recip = opool.tile([1, 1], f32)
            nc.vector.reciprocal(recip[:], denom[:])

            orow = opool.tile([1, D], f32)
            nc.scalar.activation(out=orow[:, 0:512], in_=acc_lo[:],
                                 func=mybir.ActivationFunctionType.Copy,
                                 scale=recip[:])
            nc.scalar.activation(out=orow[:, 512:1024], in_=acc_hi[:],
                                 func=mybir.ActivationFunctionType.Copy,
                                 scale=recip[:])
            nc.scalar.dma_start(out=out[b : b + 1, :], in_=orow[:])

    nc.compile()
    return nc


def _run(query, concept, trace=False, trace_kwargs=None):
    if "nc" not in _cache:
        _cache["nc"] = build_nc()
    nc = _cache["nc"]

    bf = ml_dtypes.bfloat16
    qs = (np.asarray(query, np.float32) * SCALE).astype(bf)
    cb = np.asarray(concept, np.float32).astype(bf)

    in_maps = []
    for i in range(NCORES):
        in_maps.append({
            "query": np.ascontiguousarray(qs[i * BL : (i + 1) * BL]),
            "concept": np.ascontiguousarray(
                cb[i * BL : (i + 1) * BL].reshape(BL, NCH, 128, RPP, D)),
        })
    res = run_bass_kernel_spmd(
        nc, in_maps, core_ids=list(range(NCORES)),
        trace=trace, **(trace_kwargs or {}),
    )
    out = np.concatenate([res.results[i]["out"] for i in range(NCORES)], axis=0)
    return out.astype(np.float32), res


def kernel(query: np.ndarray, concept: np.ndarray) -> np.ndarray:
    out, _ = _run(np.asarray(query, np.float32), np.asarray(concept, np.float32))
    return out


# revision 12
# speedup vs baseline: 1.2082x; 1.1620x over previous
"""Trainium2 Bass kernel for single-query attention over per-sample concepts.

    sab[b, k] = (query[b] . concept[b, k]) / sqrt(D)
    score     = softmax(sab, axis=-1)
    out[b]    = sum_k score[b, k] * concept[b, k]

Shapes: query [256, 1024] f32, concept [256, 2048, 1024] f32 -> out [256, 1024].

Sharding: pure data parallel, batch 256 split as 32 samples on each of 8
NeuronCores. The tolerance (2e-2) admits streaming concept in bf16, halving
HBM traffic to 128 MiB per core (measured end-to-end rel err ~3e-3). Host
converts to bf16 and pre-folds the 1/sqrt(D) scale into query.

Per-core dataflow (chunk = [128 part, 8, 1024] bf16 = 2 MiB, partition p
holds k-rows 8p..8p+7 of a 1024-row block; 2 chunks per sample). The score
pass (multiply by q + free-dim reduce per row) is spread across engines so
no single engine exceeds the DMA budget:
  - rows [0, ACT_RED): DVE tensor_tensor multiply at 2x (bf16) into prod,
    reduce on ACT (activation Copy + accum_out)
  - rows [ACT_RED, ACT_RED+POOL_RED): multiply in the same TT, reduce on
    GPSIMD (tensor_scalar + accum_out)
  - remaining rows: fused scalar_tensor_tensor (mult+reduce) on DVE at 1x
  - one ACT Exp per sample: scols [128,16] -> ecols bf16 + denominator
    accum; PE then runs the sample's 32 weighted-sum matmuls back-to-back
    (keeps the PE p-state warm), accumulating acc[1,512] x2 in PSUM
  - denom = ones.T @ ered matmul; DVE reciprocal; DVE tensor_scalar
    normalizes acc into orow; DMA out row.
"""

import numpy as np
import ml_dtypes
from contextlib import ExitStack

import concourse.bacc as bacc
import concourse.tile as tile
from concourse import mybir
from concourse.bass_utils import run_bass_kernel_spmd

B, K, D = 256, 2048, 1024
NCORES = 8
BL = B // NCORES          # 32 samples per core
RPP = 8                   # k-rows per partition within a chunk
CH_ROWS = 128 * RPP       # 1024 k-rows per chunk
NCH = K // CH_ROWS        # 2 chunks per sample
SCALE = 1.0 / float(np.sqrt(D))

ACT_RED = 5               # rows/chunk reduced on ACT
POOL_RED = 0              # rows/chunk reduced on GPSIMD
# rows [ACT_RED+POOL_RED, RPP) use fused STT on DVE

_cache = {}


def build_nc(bl=BL, act_red=None, pool_red=None):
    a = ACT_RED if act_red is None else act_red
    z = POOL_RED if pool_red is None else pool_red
    m = a + z                 # rows multiplied via TT (prod materialized)
    nc = bacc.Bacc("TRN2", target_bir_lowering=False, debug=False,
                   num_devices=NCORES)
    f32 = mybir.dt.float32
    bf16 = mybir.dt.bfloat16

    q = nc.dram_tensor("query", [bl, D], bf16, kind="ExternalInput")
    c = nc.dram_tensor("concept", [bl, NCH, 128, RPP, D], bf16,
                       kind="ExternalInput")
    out = nc.dram_tensor("out", [bl, D], f32, kind="ExternalOutput")

    with tile.TileContext(nc) as tc, ExitStack() as ctx:
        cpool = ctx.enter_context(tc.tile_pool(name="c", bufs=6))
        prpool = ctx.enter_context(tc.tile_pool(name="pr", bufs=2))
        stpool = ctx.enter_context(tc.tile_pool(name="st", bufs=2))
        qpool = ctx.enter_context(tc.tile_pool(name="q", bufs=3))
        scpool = ctx.enter_context(tc.tile_pool(name="sc", bufs=2))
        ecpool = ctx.enter_context(tc.tile_pool(name="ec", bufs=2))
        erpool = ctx.enter_context(tc.tile_pool(name="er", bufs=2))
        onepool = ctx.enter_context(tc.tile_pool(name="one", bufs=1))
        opool = ctx.enter_context(tc.tile_pool(name="o", bufs=4))
        plo = ctx.enter_context(tc.tile_pool(name="plo", bufs=2, space="PSUM"))
        phi = ctx.enter_context(tc.tile_pool(name="phi", bufs=2, space="PSUM"))
        dpool = ctx.enter_context(tc.tile_pool(name="dn", bufs=2, space="PSUM"))

        ones = onepool.tile([128, 1], f32)
        nc.vector.memset(ones[:], 1.0)

        for b in range(bl):
            qrow = qpool.tile([1, D], bf16)
            nc.sync.dma_start(out=qrow[:], in_=q[b : b + 1, :])
            qb = qpool.tile([128, D], bf16)
            nc.gpsimd.partition_broadcast(qb[:], qrow[:])

            scols = scpool.tile([128, NCH * RPP], f32)
            ecols = ecpool.tile([128, NCH * RPP], bf16)
            ered = erpool.tile([128, 1], f32)
            acc_lo = plo.tile([1, 512], f32)
            acc_hi = phi.tile([1, 512], f32)
            cts = []

            for h in range(NCH):
                ct = cpool.tile([128, RPP, D], bf16, name="ct")
                dma_eng = nc.sync if h % 2 == 0 else nc.scalar
                dma_eng.dma_start(out=ct[:], in_=c[b, h])
                cts.append(ct)

                prod = prpool.tile([128, m, D], bf16, name="prod") if m else None
                if m:
                    nc.vector.tensor_tensor(
                        out=prod[:], in0=ct[:, 0:m],
                        in1=qb[:].unsqueeze(1).broadcast_to([128, m, D]),
                        op=mybir.AluOpType.mult)

                for j in range(RPP):
                    col = h * RPP + j
                    acc = scols[:, col : col + 1]
                    if j < m:
                        sl = prod[:, j]
                        if j < a:
                            nc.scalar.activation(
                                out=sl, in_=sl,
                                func=mybir.ActivationFunctionType.Copy,
                                accum_out=acc,
                            )
                        else:
                            nc.gpsimd.tensor_scalar(
                                out=sl, in0=sl, scalar1=1.0, scalar2=0.0,
                                op0=mybir.AluOpType.mult,
                                op1=mybir.AluOpType.add,
                                accum_out=acc,
                            )
                    else:
                        scr = stpool.tile([128, D], bf16, name="scr")
                        nc.vector.scalar_tensor_tensor(
                            out=scr[:],
                            in0=ct[:, j], scalar=1.0, in1=qb[:],
                            op0=mybir.AluOpType.mult,
                            op1=mybir.AluOpType.mult,
                            accum_out=acc,
                        )

            nc.scalar.activation(
                out=ecols[:], in_=scols[:],
                func=mybir.ActivationFunctionType.Exp,
                accum_out=ered[:],
            )

            for h in range(NCH):
                for j in range(RPP):
                    col = h * RPP + j
                    e = ecols[:, col : col + 1]
                    nc.tensor.matmul(acc_lo[:], e, cts[h][:, j, 0:512],
                                     start=(col == 0),
                                     stop=(col == NCH * RPP - 1))
                    nc.tensor.matmul(acc_hi[:], e, cts[h][:, j, 512:1024],
                                     start=(col == 0),
                                     stop=(col == NCH * RPP - 1))

            denom = dpool.tile([1, 1], f32)
            nc.tensor.matmul(denom[:], ones[:], ered[:], start=True, stop=True)

            recip = opool.tile([1, 1], f32)
            nc.vector.reciprocal(recip[:], denom[:])

            orow = opool.tile([1, D], f32)
            nc.vector.tensor_scalar(out=orow[:, 0:512], in0=acc_lo[:],
                                    scalar1=recip[:], scalar2=None,
                                    op0=mybir.AluOpType.mult)
            nc.vector.tensor_scalar(out=orow[:, 512:1024], in0=acc_hi[:],
                                    scalar1=recip[:], scalar2=None,
                                    op0=mybir.AluOpType.mult)
            nc.scalar.dma_start(out=out[b : b + 1, :], in_=orow[:])

    nc.compile()
    return nc


def _run(query, concept, trace=False, trace_kwargs=None):
    if "nc" not in _cache:
        _cache["nc"] = build_nc()
    nc = _cache["nc"]

    bf = ml_dtypes.bfloat16
    qs = (np.asarray(query, np.float32) * SCALE).astype(bf)
    cb = np.asarray(concept, np.float32).astype(bf)

    in_maps = []
    for i in range(NCORES):
        in_maps.append({
            "query": np.ascontiguousarray(qs[i * BL : (i + 1) * BL]),
            "concept": np.ascontiguousarray(
                cb[i * BL : (i + 1) * BL].reshape(BL, NCH, 128, RPP, D)),
        })
    res = run_bass_kernel_spmd(
        nc, in_maps, core_ids=list(range(NCORES)),
        trace=trace, **(trace_kwargs or {}),
    )
    out = np.concatenate([res.results[i]["out"] for i in range(NCORES)], axis=0)
    return out.astype(np.float32), res


def kernel(query: np.ndarray, concept: np.ndarray) -> np.ndarray:
    out, _ = _run(np.asarray(query, np.float32), np.asarray(concept, np.float32))
    return out
